# revision 1
# baseline (speedup 1.0000x reference)
"""Multi-head causal attention with RoPE for TRN2, 8 NeuronCores.

Problem: B=2, T=2048, D=2048, 16 heads x head_dim 128, fp32.
  qkv = x @ Wqkv.T + bqkv ; RoPE(q, k) interleaved-pairs; causal softmax attention;
  out = attn_out @ Wo.T + bo.

Sharding: core c in 0..7 -> (batch b = c//4, head-group g = c%4 of 4 heads).
Each core computes its batch's partial output (its 4 heads' contribution through
the out-projection); host sums the 4 group partials per batch and adds bo.

Per-core kernel (all matmuls fp32r: full PE speed, ~1e-3 scale-relative rounding):
  Phase A: qkvT projection. q,k produced transposed [d, t] with head_dim rows
    de-interleaved (even dims then odd dims) so RoPE's rotate-half becomes a
    half-swap along partitions, done via a permutation matmul on PE. RoPE is
    applied during the PSUM drain. k lands directly in persistent SBUF tiles
    (no DRAM roundtrip); q and v roundtrip through DRAM.
  Phase B: per 512-wide q-tile, per head:
    S^T[k,q] = kT.T @ qT on PE, exp on ACT (scale 1/sqrt(dh) folded in),
    causal masks on diagonal blocks (DVE), PV accumulate O^T[d,q] (PE),
    denominator by ones-matmul (PE), reciprocal + partition-broadcast +
    normalize (DVE/GPSIMD).
  Phase C (interleaved per q-tile): final[t,o] += O^T_h[:,t].T @ WoT_h[:,o].
"""
import os
import sys

for _p in ("/opt/trn_rl_repo", "/root/.axon_site/_ro/trn_rl_repo"):
    if os.path.isdir(_p) and _p not in sys.path:
        sys.path.insert(0, _p)

import numpy as np

import concourse.bacc as bacc
import concourse.mybir as mybir
import concourse.tile as tile
from concourse.bass_utils import run_bass_kernel_spmd

dt = mybir.dt
AF = mybir.ActivationFunctionType

B = 2
T = 2048
D = 2048
NH = 16
HD = 128
ROPE_BASE = 10000.0
N_CORES = 8
GROUPS = 4          # head-groups (tensor-parallel axis)
HPG = NH // GROUPS  # heads per group = 4
FQK = HPG * HD      # 512: q (or k) feature cols per core
FV = HPG * HD       # 512
QT = 512            # q-tile width in attention
NQT = T // QT       # 4
NKC = T // 128      # 16 k-chunks
NCC = D // 128      # 16 contraction chunks
TB = 512            # phase-A t-block
NTB = T // TB       # 4
SCALE = 1.0 / float(np.sqrt(HD))


def build(loop=1):
    """Emit the per-core BIR program (identical for all 8 cores)."""
    import contextlib

    nc = bacc.Bacc("TRN2", target_bir_lowering=False, debug=False)

    xT_d = nc.dram_tensor("xT", [D, T], dt.float32r, kind="ExternalInput")
    wqp_d = nc.dram_tensor("wqpack", [8, 128, NCC * 128], dt.float32r,
                           kind="ExternalInput")
    wvp_d = nc.dram_tensor("wvpack", [128, NCC * FV], dt.float32r,
                           kind="ExternalInput")
    woT_d = nc.dram_tensor("woT", [FV, D], dt.float32r, kind="ExternalInput")
    cos_d = nc.dram_tensor("cosT", [HD, T], dt.float16, kind="ExternalInput")
    sin_d = nc.dram_tensor("sinT", [HD, T], dt.float16, kind="ExternalInput")
    mask_d = nc.dram_tensor("masks", [4, HD, QT], dt.float32, kind="ExternalInput")
    bqk_d = nc.dram_tensor("bqk", [2 * FQK, 1], dt.float32, kind="ExternalInput")
    bv_d = nc.dram_tensor("bvb", [HD, FV], dt.float32, kind="ExternalInput")
    ones_d = nc.dram_tensor("ones", [HD, 1], dt.float32r, kind="ExternalInput")
    perm_d = nc.dram_tensor("perm", [HD, HD], dt.float32r, kind="ExternalInput")
    out_d = nc.dram_tensor("outp", [T, D], dt.float32, kind="ExternalOutput")

    with tile.TileContext(nc, pool_alloc_mode="queue") as tc:
        with (
            tc.For_i(0, loop, 1) if loop > 1 else contextlib.nullcontext(),
            tc.tile_pool(name="dram", bufs=1, space="DRAM") as dramp,
            tc.tile_pool(name="kres", bufs=1) as kres,
        ):
            qT_tbs, vN_tbs = [], []
            for tb in range(NTB):
                qT_tb = dramp.tile([FQK, TB], dt.float32r, tag=f"qT{tb}",
                                   name=f"qT_{tb}")
                qT_tbs.append(qT_tb)
                vN_tb = dramp.tile([TB, FV], dt.float32r, tag=f"vN{tb}",
                                   name=f"vN_{tb}")
                vN_tbs.append(vN_tb)

            k_rs = []
            for h in range(HPG):
                k_r = kres.tile([HD, T], dt.float32r, tag=f"kr{h}", name=f"kr_{h}")
                k_rs.append(k_r)

            # -------- Phase A: qkv projection + RoPE on q,k (during drain) --------
            with (
                tc.tile_pool(name="wq", bufs=1) as wpool,
                tc.tile_pool(name="xb", bufs=2) as xpool,
                tc.tile_pool(name="adr", bufs=2) as adrain,
                tc.tile_pool(name="arope", bufs=2) as arope,
                tc.tile_pool(name="abias", bufs=1) as abias,
                tc.tile_pool(name="aps", bufs=2, space="PSUM") as aps,
                tc.tile_pool(name="rps", bufs=2, space="PSUM") as rps,
            ):
                bqk_sb = abias.tile([128, 8, 1], dt.float32)
                nc.scalar.dma_start(
                    out=bqk_sb, in_=bqk_d.ap().rearrange("(f p) o -> p f o", p=128)
                )
                bv_sb = abias.tile([HD, FV], dt.float32)
                nc.scalar.dma_start(out=bv_sb, in_=bv_d.ap())
                cos_t = abias.tile([HD, T], dt.float16)
                sin_t = abias.tile([HD, T], dt.float16)
                nc.scalar.dma_start(out=cos_t, in_=cos_d.ap())
                nc.scalar.dma_start(out=sin_t, in_=sin_d.ap())
                perm_t = abias.tile([HD, HD], dt.float32r)
                nc.scalar.dma_start(out=perm_t, in_=perm_d.ap())

                def load_xb(tb):
                    tsl = slice(tb * TB, (tb + 1) * TB)
                    xbl = []
                    for cc in range(NCC):
                        xb_c = xpool.tile(
                            [128, TB], dt.float32r, tag=f"xb{cc}",
                            name=f"xb_{tb}_{cc}", bufs=(1 if cc >= 14 else 2),
                        )
                        nc.sync.dma_start(
                            out=xb_c,
                            in_=xT_d.ap()[cc * 128:(cc + 1) * 128, tsl],
                        )
                        xbl.append(xb_c)
                    return xbl

                # first t-block's activations win the sync queue
                xb0 = load_xb(0)

                # qk weights as 8 column-blocks, split across both HWDGE
                # queues (evens+v on scalar, odds on sync behind xb0)
                wq_blocks = [None] * 8
                for fb, eng in ((0, nc.scalar), (4, nc.scalar), (1, nc.sync),
                                (5, nc.sync), (2, nc.scalar), (6, nc.scalar),
                                (3, nc.sync), (7, nc.sync)):
                    wq_b = wpool.tile([128, NCC, 128], dt.float32r, tag=f"wq{fb}",
                                      name=f"wq_{fb}")
                    eng.dma_start(
                        out=wq_b,
                        in_=wqp_d.ap()[fb].rearrange("p (cc f) -> p cc f", f=128),
                    )
                    wq_blocks[fb] = wq_b
                wv_b = wpool.tile([128, NCC, FV], dt.float32r)
                nc.scalar.dma_start(
                    out=wv_b,
                    in_=wvp_d.ap().rearrange("p (cc f) -> p cc f", f=FV),
                )
                for tb in range(NTB):
                    tsl = slice(tb * TB, (tb + 1) * TB)
                    xb = xb0 if tb == 0 else load_xb(tb)
                    # q,k: transposed [f, t]; RoPE during drain; k -> SBUF resident
                    for f in (0, 4, 1, 5, 2, 6, 3, 7):
                        ps = aps.tile([128, TB], dt.float32)
                        for cc in range(NCC):
                            nc.tensor.matmul(
                                ps,
                                wq_blocks[f][:, cc, :],
                                xb[cc],
                                start=(cc == 0),
                                stop=(cc == NCC - 1),
                            )
                        s1 = arope.tile([128, TB], dt.float32r, tag="s1")
                        nc.vector.tensor_scalar_add(s1, ps, bqk_sb[:, f, :])
                        rot_ps = rps.tile([128, TB], dt.float32)
                        nc.tensor.matmul(rot_ps, perm_t, s1, start=True, stop=True)
                        nc.vector.tensor_mul(out=s1, in0=s1, in1=cos_t[:, tsl])
                        nc.vector.tensor_mul(out=rot_ps, in0=rot_ps, in1=sin_t[:, tsl])
                        if f < 4:  # q -> DRAM roundtrip
                            dr = adrain.tile([128, TB], dt.float32r, tag="adr")
                            nc.vector.tensor_add(out=dr, in0=s1, in1=rot_ps)
                            nc.sync.dma_start(
                                out=qT_tbs[tb][f * 128:(f + 1) * 128, :], in_=dr,
                            )
                        else:      # k -> persistent SBUF
                            nc.vector.tensor_add(
                                out=k_rs[f - 4][:, tsl], in0=s1, in1=rot_ps
                            )
                    # v: natural output [t, d]
                    for ts4 in range(TB // 128):
                        ps = aps.tile([128, FV], dt.float32)
                        for cc in range(NCC):
                            nc.tensor.matmul(
                                ps,
                                xb[cc][:, ts4 * 128:(ts4 + 1) * 128],
                                wv_b[:, cc, :],
                                start=(cc == 0),
                                stop=(cc == NCC - 1),
                            )
                        dr = adrain.tile([128, FV], dt.float32r, tag="adr")
                        nc.vector.tensor_add(dr, ps, bv_sb)
                        nc.sync.dma_start(
                            out=vN_tbs[tb][ts4 * 128:(ts4 + 1) * 128, :],
                            in_=dr,
                        )

            # -------- Phase B + C: attention, out-proj per q-tile --------
            with (
                tc.tile_pool(name="bsing", bufs=1) as bsing,
                tc.tile_pool(name="qt", bufs=2) as qtp,
                tc.tile_pool(name="vt", bufs=1) as vtp,
                tc.tile_pool(name="osb", bufs=2) as osbp,
                tc.tile_pool(name="pt", bufs=4) as ptp,
                tc.tile_pool(name="bsmall", bufs=2) as bsmall,
                tc.tile_pool(name="wo", bufs=1) as wop,
                tc.tile_pool(name="cdr", bufs=3) as cdrain,
                tc.tile_pool(name="ps_s", bufs=3, space="PSUM") as ps_s,
                tc.tile_pool(name="ps_o", bufs=2, space="PSUM") as ps_o,
                tc.tile_pool(name="ps_l", bufs=1, space="PSUM") as ps_l,
                tc.tile_pool(name="cps", bufs=2, space="PSUM") as cps,
            ):
                mask_t = bsing.tile([HD, 4, QT], dt.float32)
                nc.sync.dma_start(out=mask_t, in_=mask_d.ap().transpose([1, 0, 2]))
                ones_t = bsing.tile([HD, 1], dt.float32r)
                nc.scalar.dma_start(out=ones_t, in_=ones_d.ap())

                # first q-tile's q loads win the queue; v chunks tb-major,
                # alternating the two HWDGE queues
                q_t0s = []
                for h in range(HPG):
                    q_t = qtp.tile([HD, QT], dt.float32r, tag=f"qt{h}",
                                   name=f"qt_0_{h}")
                    nc.scalar.dma_start(out=q_t, in_=qT_tbs[0][h * HD:(h + 1) * HD, :])
                    q_t0s.append(q_t)
                v_ts = []
                for h in range(HPG):
                    v_t = vtp.tile([128, NKC, HD], dt.float32r, tag=f"v{h}",
                                   name=f"v_{h}")
                    v_ts.append(v_t)
                qi = 0
                for tb in range(NTB):
                    for h in range(HPG):
                        eng = nc.scalar if (qi % 2 == 0) else nc.sync
                        qi += 1
                        eng.dma_start(
                            out=v_ts[h][:, 4 * tb:4 * (tb + 1), :],
                            in_=vN_tbs[tb][:, h * HD:(h + 1) * HD].rearrange(
                                "(c p) d -> p c d", p=128
                            ),
                        )

                wo_sb = wop.tile([128, HPG, D], dt.float32r)
                nc.sync.dma_start(
                    out=wo_sb, in_=woT_d.ap().rearrange("(hh p) o -> p hh o", p=128)
                )

                def emit_cproj(pj, o_hs, tts, on_act=False):
                    # out-projection tiles (tt in tts) for q-tile pj
                    for tt in tts:
                        for oo in range(D // QT):
                            ps = cps.tile([128, QT], dt.float32,
                                          name=f"cps_{pj}_{tt}_{oo}", tag="cps")
                            for h in range(HPG):
                                nc.tensor.matmul(
                                    ps,
                                    o_hs[h][:, tt * 128:(tt + 1) * 128],
                                    wo_sb[:, h, oo * QT:(oo + 1) * QT],
                                    start=(h == 0), stop=(h == HPG - 1),
                                )
                            dr = cdrain.tile([128, QT], dt.float32,
                                             name=f"cdr_{pj}_{tt}_{oo}", tag="cdr")
                            if on_act:
                                nc.scalar.copy(out=dr, in_=ps)
                            else:
                                nc.vector.tensor_copy(out=dr, in_=ps)
                            nc.sync.dma_start(
                                out=out_d.ap()[
                                    pj * QT + tt * 128: pj * QT + (tt + 1) * 128,
                                    oo * QT:(oo + 1) * QT,
                                ],
                                in_=dr,
                            )

                prev_o = None
                for j in range(NQT):
                    nkc = 4 * (j + 1)
                    o_heads = []
                    for h in range(HPG):
                        if j == 0:
                            q_t = q_t0s[h]
                        else:
                            q_t = qtp.tile([HD, QT], dt.float32r, tag=f"qt{h}",
                                           name=f"qt_{j}_{h}")
                            nc.scalar.dma_start(
                                out=q_t, in_=qT_tbs[j][h * HD:(h + 1) * HD, :]
                            )
                        o_head_tile = osbp.tile([HD, QT], dt.float32r, tag=f"osb{h}",
                                                name=f"osb_{j}_{h}")
                        o_heads.append(o_head_tile)
                        psum_o = ps_o.tile([HD, QT], dt.float32)
                        psum_l = ps_l.tile([1, QT], dt.float32)

                        def col0(kc):
                            m = kc - 4 * j
                            if m <= 0:
                                return 0
                            return 128 if m == 1 else 256

                        def s_matmul(kc):
                            c0 = col0(kc)
                            psum_s = ps_s.tile(
                                [128, QT], dt.float32,
                                name=f"s_{j}_{h}_{kc}", tag="psum_s",
                            )
                            nc.tensor.matmul(
                                psum_s[:, c0:],
                                k_rs[h][:, kc * 128:(kc + 1) * 128],
                                q_t[:, c0:],
                                start=True, stop=True,
                            )
                            return psum_s

                        s_next = s_matmul(0)
                        for kc in range(nkc):
                            psum_s = s_next
                            if kc + 1 < nkc:
                                s_next = s_matmul(kc + 1)
                            c0 = col0(kc)
                            pt = ptp.tile([128, QT], dt.float32r)
                            nc.scalar.activation(
                                out=pt[:, c0:], in_=psum_s[:, c0:],
                                func=AF.Exp, scale=SCALE,
                            )
                            m = kc - 4 * j
                            if m >= 0:
                                nc.vector.tensor_mul(
                                    out=pt[:, c0:], in0=pt[:, c0:],
                                    in1=mask_t[:, m, c0:],
                                )
                            nc.tensor.matmul(
                                psum_o[:, c0:], v_ts[h][:, kc, :], pt[:, c0:],
                                start=(kc == 0), stop=(kc == nkc - 1),
                            )
                            nc.tensor.matmul(
                                psum_l[:, c0:], ones_t, pt[:, c0:],
                                start=(kc == 0), stop=(kc == nkc - 1),
                            )
                        recip = bsmall.tile([1, QT], dt.float32, tag="recip")
                        nc.vector.reciprocal(out=recip, in_=psum_l)
                        bcast = bsmall.tile([128, QT], dt.float32, tag="bcast")
                        nc.gpsimd.partition_broadcast(bcast, recip)
                        nc.vector.tensor_mul(
                            out=o_heads[h], in0=psum_o, in1=bcast
                        )
                        # interleave previous q-tile's out-projection
                        if prev_o is not None:
                            emit_cproj(j - 1, prev_o, [h])
                    prev_o = o_heads
                emit_cproj(NQT - 1, prev_o, list(range(QT // 128)), on_act=True)
    nc.compile()
    return nc


# ---------------------------------------------------------------------------
# Host side
# ---------------------------------------------------------------------------

_DEINT = np.concatenate([np.arange(0, HD, 2), np.arange(1, HD, 2)])  # de-interleave


def _rope_tables():
    half = HD // 2
    inv_freq = 1.0 / (ROPE_BASE ** (np.arange(half, dtype=np.float64) / half))
    t = np.arange(T, dtype=np.float64)
    fr = t[None, :] * inv_freq[:, None]          # (64, T)
    cos = np.concatenate([np.cos(fr), np.cos(fr)], axis=0).astype(np.float16)
    sin = np.concatenate([-np.sin(fr), np.sin(fr)], axis=0).astype(np.float16)
    return cos, sin


def _masks():
    m = np.zeros((4, HD, QT), dtype=np.float32)
    kk = np.arange(HD)[:, None]
    qq = np.arange(QT)[None, :]
    for i in range(4):
        m[i] = (kk <= qq - 128 * i).astype(np.float32)
    return m


def _perm():
    p = np.zeros((HD, HD), dtype=np.float32)
    half = HD // 2
    for i in range(half):
        p[i + half, i] = 1.0   # rot[m<64]  = s1[m+64]
        p[i, i + half] = 1.0   # rot[m>=64] = s1[m-64]
    return p


def make_in_maps(x, Wqkv, bqkv, Wo, bo):
    cos, sin = _rope_tables()
    masks = _masks()
    ones = np.ones((HD, 1), dtype=np.float32)
    perm = _perm()

    Wq = Wqkv[0 * D:1 * D]
    Wk = Wqkv[1 * D:2 * D]
    Wv = Wqkv[2 * D:3 * D]
    bq = bqkv[0 * D:1 * D]
    bk = bqkv[1 * D:2 * D]
    bv = bqkv[2 * D:3 * D]

    in_maps = []
    for c in range(N_CORES):
        b, g = divmod(c, GROUPS)
        hsl = slice(g * HPG * HD, (g + 1) * HPG * HD)
        # de-interleaved row order for q,k heads of this group
        rows = np.arange(g * HPG * HD, (g + 1) * HPG * HD).reshape(HPG, HD)
        rows = rows[:, _DEINT].reshape(-1)

        wq = Wq[rows]                       # (512, D)
        wk = Wk[rows]
        wv = Wv[hsl]                        # natural order
        wqkT = np.concatenate([wq, wk], axis=0).T.astype(np.float32)  # (D, 1024)
        # packed [fb, p, cc*f]: per-partition contiguous DMA rows
        wqpack = np.ascontiguousarray(
            wqkT.reshape(NCC, 128, 8, 128)      # (cc, p, fb, f)
                .transpose(2, 1, 0, 3)           # (fb, p, cc, f)
                .reshape(8, 128, NCC * 128)
        )
        wvT = wv.T.astype(np.float32)            # (D, 512)
        wvpack = np.ascontiguousarray(
            wvT.reshape(NCC, 128, FV).transpose(1, 0, 2).reshape(128, NCC * FV)
        )
        woT = np.ascontiguousarray(Wo[:, hsl].T.astype(np.float32))  # (512, D)

        bqk = np.concatenate([bq[rows], bk[rows]]).astype(np.float32)[:, None]
        bvb = np.broadcast_to(bv[hsl].astype(np.float32), (HD, FV)).copy()

        xT = np.ascontiguousarray(np.asarray(x[b]).T.astype(np.float32))  # (D, T)

        in_maps.append({
            "xT": xT,
            "wqpack": wqpack,
            "wvpack": wvpack,
            "woT": woT,
            "cosT": cos,
            "sinT": sin,
            "masks": masks,
            "bqk": bqk,
            "bvb": bvb,
            "ones": ones,
            "perm": perm,
        })
    return in_maps


_NC_CACHE = {}


def _get_nc(loop=1):
    if loop not in _NC_CACHE:
        _NC_CACHE[loop] = build(loop=loop)
    return _NC_CACHE[loop]


def kernel(x, Wqkv, bqkv, Wo, bo):
    x = np.asarray(x)
    Wqkv = np.asarray(Wqkv)
    bqkv = np.asarray(bqkv)
    Wo = np.asarray(Wo)
    bo = np.asarray(bo)

    nc = _get_nc()
    in_maps = make_in_maps(x, Wqkv, bqkv, Wo, bo)
    res = run_bass_kernel_spmd(nc, in_maps, core_ids=list(range(N_CORES)))

    out = np.zeros((B, T, D), dtype=np.float32)
    for c in range(N_CORES):
        b = c // GROUPS
        out[b] += res.results[c]["outp"]
    out += bo.astype(np.float32)[None, None, :]
    return out



# revision 14
# speedup vs baseline: 1.2206x; 1.2206x over previous
"""Multi-head causal attention with RoPE for TRN2, 8 NeuronCores.

Problem: B=2, T=2048, D=2048, 16 heads x head_dim 128, fp32 in/out.
  qkv = x @ Wqkv.T + bqkv ; RoPE(q,k) interleaved-pairs; causal softmax attention;
  out = attn_out @ Wo.T + bo.

Sharding: core c in 0..7 -> (batch b = c//4, head-group g = c%4 of 4 heads).
Each core computes its batch's partial output (its 4 heads' contribution through
the out-projection); host sums the 4 group partials per batch and adds bo.

v2 design (vs the fp32r DRAM-roundtrip baseline):
  - bf16 datapath everywhere on PE (weights, x, q, k, v, pt, O, Wo); fp32 PSUM.
  - No DRAM roundtrips: q/k/v drain from PSUM directly into SBUF in the exact
    layout attention needs (q tiles ARE [head, 512-q-tile] blocks; v tiles ARE
    [128 kpos, kc, head*128..] slices of the projection drain).
  - Phase A (projection) and attention pipelined per 512-token t-block:
    causal q-tile j only needs k,v up to (j+1)*512, i.e. t-blocks <= j.
    A-unit matmuls are drip-fed into attention's PE idle slots (attention alone
    is ACT(exp)-bound; PE has ~200ns/chunk spare).
  - RoPE rotate-half via partition-offset DVE muls (no PE perm matmul).
  - Softmax denominator for free: v is stored with a ones-column appended
    ([128, kc, 129]); attention output is computed TRANSPOSED per 128-wide
    q-sub-block (stationary = pt slice, moving = v_aug) so each PV matmul
    emits [q, 128 d + denom] in one pass. Normalization is then a per-partition
    tensor_scalar_mul with reciprocal_approx_fast on [128,4] (cheap), and the
    [q,d] -> [d,q] flip for the out-projection is 4 tiny PE transposes.
"""
import os
import sys

for _p in ("/opt/trn_rl_repo", "/root/.axon_site/_ro/trn_rl_repo"):
    if os.path.isdir(_p) and _p not in sys.path:
        sys.path.insert(0, _p)

import numpy as np
import ml_dtypes

import concourse.bacc as bacc
import concourse.mybir as mybir
import concourse.tile as tile
from concourse.bass_utils import run_bass_kernel_spmd

dt = mybir.dt
AF = mybir.ActivationFunctionType
BF16 = ml_dtypes.bfloat16

B = 2
T = 2048
D = 2048
NH = 16
HD = 128
ROPE_BASE = 10000.0
N_CORES = 8
GROUPS = 4          # head-groups (tensor-parallel axis)
HPG = NH // GROUPS  # heads per group = 4
QT = 512            # q-tile width in attention
NQT = T // QT       # 4
NCC = D // 128      # 16 contraction chunks
TB = 512            # phase-A t-block == QT
NTB = T // TB       # 4
SCALE = 1.0 / float(np.sqrt(HD))


def build(loop=1):
    """Emit the per-core BIR program (identical for all 8 cores)."""
    import contextlib

    nc = bacc.Bacc("TRN2", target_bir_lowering=False, debug=False)

    xp_d = nc.dram_tensor("xpack", [128, NCC, T], dt.bfloat16, kind="ExternalInput")
    wqp_d = nc.dram_tensor("wqpack", [8, 128, NCC * 128], dt.bfloat16,
                           kind="ExternalInput")
    wvp_d = nc.dram_tensor("wvpack", [128, NCC * 512], dt.bfloat16,
                           kind="ExternalInput")
    woT_d = nc.dram_tensor("woT", [HPG * HD, D], dt.bfloat16, kind="ExternalInput")
    cos_d = nc.dram_tensor("cosT", [HD, T], dt.float16, kind="ExternalInput")
    sin_d = nc.dram_tensor("sinT", [HD, T], dt.float16, kind="ExternalInput")
    mask_d = nc.dram_tensor("mask", [128, 128], dt.float32, kind="ExternalInput")
    bqk_d = nc.dram_tensor("bqk", [8 * 128, 1], dt.float32, kind="ExternalInput")
    bv_d = nc.dram_tensor("bvb", [HD, 512], dt.float32, kind="ExternalInput")
    idn_d = nc.dram_tensor("idn", [128, 128], dt.bfloat16, kind="ExternalInput")
    out_d = nc.dram_tensor("outp", [T, D], dt.bfloat16, kind="ExternalOutput")

    with tile.TileContext(nc, pool_alloc_mode="queue") as tc:
        with contextlib.ExitStack() as ctx:
            if loop > 1:
                ctx.enter_context(tc.For_i(0, loop, 1))
            P = lambda *a, **kw: ctx.enter_context(tc.tile_pool(*a, **kw))
            kres = P(name="kres", bufs=1)
            wpool = P(name="wq", bufs=1)
            xpool = P(name="xb", bufs=1)
            s1p = P(name="s1p", bufs=2)
            rotp = P(name="rotp", bufs=2)
            csts = P(name="csts", bufs=1)
            qres = P(name="qres", bufs=1)
            vres = P(name="vres", bufs=1)
            ptp = P(name="ptp", bufs=4)
            osbp = P(name="osb", bufs=4)
            ohp = P(name="ohp", bufs=2)
            rcpp = P(name="rcpp", bufs=2)
            cdrp = P(name="cdr", bufs=2)
            aps = P(name="aps", bufs=2, space="PSUM")
            ps_s = P(name="ps_s", bufs=2, space="PSUM")
            poaA = P(name="poaA", bufs=1, space="PSUM")
            poaB = P(name="poaB", bufs=1, space="PSUM")
            ptrp = P(name="ptr", bufs=1, space="PSUM")
            cps = P(name="cps", bufs=1, space="PSUM")
            # ---------------- constant / weight loads (scalar HWDGE) --------
            wq_blocks = [None] * 8
            def load_wq(f):
                wq_b = wpool.tile([128, NCC, 128], dt.bfloat16, tag=f"wq{f}",
                                  name=f"wq_{f}")
                nc.scalar.dma_start(
                    out=wq_b,
                    in_=wqp_d.ap()[f].rearrange("p (cc f) -> p cc f", f=128),
                )
                wq_blocks[f] = wq_b

            load_wq(0)
            load_wq(4)
            bqk_sb = csts.tile([128, 8, 1], dt.float32)
            nc.scalar.dma_start(
                out=bqk_sb, in_=bqk_d.ap().rearrange("(f p) o -> p f o", p=128)
            )
            cos_t = csts.tile([HD, T], dt.float16)
            sin_t = csts.tile([HD, T], dt.float16)
            nc.scalar.dma_start(out=cos_t, in_=cos_d.ap())
            nc.scalar.dma_start(out=sin_t, in_=sin_d.ap())
            load_wq(1)
            load_wq(5)
            bv_sb = csts.tile([HD, 512], dt.float32)
            nc.scalar.dma_start(out=bv_sb, in_=bv_d.ap())
            mask_t = csts.tile([128, 128], dt.float32)
            nc.scalar.dma_start(out=mask_t, in_=mask_d.ap())
            idn_t = csts.tile([128, 128], dt.bfloat16)
            nc.scalar.dma_start(out=idn_t, in_=idn_d.ap())
            load_wq(2)
            load_wq(6)
            load_wq(3)
            load_wq(7)
            wv_b = wpool.tile([128, NCC, 512], dt.bfloat16)
            nc.scalar.dma_start(
                out=wv_b, in_=wvp_d.ap().rearrange("p (cc f) -> p cc f", f=512),
            )
            wo_sb = wpool.tile([128, HPG, D], dt.bfloat16)
            nc.scalar.dma_start(
                out=wo_sb, in_=woT_d.ap().rearrange("(hh p) o -> p hh o", p=128)
            )

            # ---------------- persistent q/k/v ------------------------------
            k_rs = [kres.tile([HD, T], dt.bfloat16, tag=f"kr{h}", name=f"kr_{h}")
                    for h in range(HPG)]
            q_ts = {}
            for tb in range(NTB):
                for h in range(HPG):
                    q_ts[(tb, h)] = qres.tile([HD, QT], dt.bfloat16,
                                              tag=f"q{tb}_{h}", name=f"q_{tb}_{h}")
            v_aug = [vres.tile([128, 4 * NTB, 129], dt.bfloat16, tag=f"v{h}",
                               name=f"v_{h}") for h in range(HPG)]
            for h in range(HPG):
                nc.gpsimd.memset(v_aug[h][:, :, 128], 1.0)

            # ---------------- x slab loads (sync HWDGE) ---------------------
            x_slabs = {}

            def load_x(tb):
                tsl = slice(tb * TB, (tb + 1) * TB)
                if tb == 0:
                    parts = []
                    for qr in range(4):
                        xq = xpool.tile([128, 4, TB], dt.bfloat16, tag=f"x0q{qr}",
                                        name=f"x0_{qr}", bufs=1)
                        nc.sync.dma_start(
                            out=xq, in_=xp_d.ap()[:, 4 * qr:4 * (qr + 1), tsl]
                        )
                        parts.append(xq)
                    x_slabs[0] = ("quads", parts)
                else:
                    xs = xpool.tile([128, NCC, TB], dt.bfloat16, tag="xslab",
                                    name=f"x_{tb}")
                    nc.sync.dma_start(out=xs, in_=xp_d.ap()[:, :, tsl])
                    x_slabs[tb] = ("slab", xs)

            def x_chunk(tb, cc):
                kind, v = x_slabs[tb]
                if kind == "quads":
                    return v[cc // 4][:, cc % 4, :]
                return v[:, cc, :]

            # ---------------- phase-A unit (drip-fed into PE stream) --------
            class AUnit:
                """One projection unit: 16 accumulating matmuls + drain.

                kind 'qk': f-block f (0-3 = q heads, 4-7 = k heads), out RoPE'd.
                kind 'v': ts4 sub-block, out v_aug slices (+bias).
                """

                def __init__(self, tb, kind, idx):
                    self.tb, self.kind, self.idx = tb, kind, idx
                    self.cc = 0
                    self.ps = None

                def step(self):
                    """Emit one PE matmul; returns True when unit is done."""
                    tb, kind, idx = self.tb, self.kind, self.idx
                    if self.ps is None:
                        self.ps = aps.tile(
                            [128, TB], dt.float32, tag="aps",
                            name=f"aps_{tb}_{kind}{idx}",
                        )
                    cc = self.cc
                    if kind == "qk":
                        nc.tensor.matmul(
                            self.ps, wq_blocks[idx][:, cc, :], x_chunk(tb, cc),
                            start=(cc == 0), stop=(cc == NCC - 1),
                        )
                    else:
                        nc.tensor.matmul(
                            self.ps,
                            x_chunk(tb, cc)[:, idx * 128:(idx + 1) * 128],
                            wv_b[:, cc, :],
                            start=(cc == 0), stop=(cc == NCC - 1),
                        )
                    self.cc += 1
                    if self.cc < NCC:
                        return False
                    self._drain()
                    return True

                def _drain(self):
                    tb, kind, idx = self.tb, self.kind, self.idx
                    tsl = slice(tb * TB, (tb + 1) * TB)
                    if kind == "qk":
                        f = idx
                        s1 = s1p.tile([128, TB], dt.bfloat16, tag="s1")
                        nc.vector.tensor_scalar_add(s1, self.ps, bqk_sb[:, f, :])
                        rot = rotp.tile([128, TB], dt.bfloat16, tag="rot")
                        # sin table halves are pre-swapped host-side so both
                        # SBUF inputs share a base partition (HW constraint)
                        nc.vector.tensor_mul(
                            out=rot[0:64, :], in0=s1[64:128, :],
                            in1=sin_t[64:128, tsl],
                        )
                        nc.vector.tensor_mul(
                            out=rot[64:128, :], in0=s1[0:64, :],
                            in1=sin_t[0:64, tsl],
                        )
                        nc.vector.tensor_mul(out=s1, in0=s1, in1=cos_t[:, tsl])
                        if f < 4:
                            nc.vector.tensor_add(out=q_ts[(tb, f)], in0=s1, in1=rot)
                        else:
                            nc.vector.tensor_add(
                                out=k_rs[f - 4][:, tsl], in0=s1, in1=rot
                            )
                    else:
                        kc = 4 * tb + idx
                        for h in range(HPG):
                            nc.vector.tensor_add(
                                out=v_aug[h][:, kc, 0:128],
                                in0=self.ps[:, h * 128:(h + 1) * 128],
                                in1=bv_sb[:, h * 128:(h + 1) * 128],
                            )

            # ---------------- cproj unit (out-projection, drip-fed) ---------
            class CUnit:
                """Out-projection for q-tile pj, row-block tt: 4 oo groups of
                4 accumulating matmuls, drained to one [128, 4*512] bf16 tile,
                then one DMA."""

                def __init__(self, pj, tt, o_heads):
                    self.pj, self.tt, self.o_heads = pj, tt, o_heads
                    self.step_i = 0
                    self.ps = None
                    self.dr = None

                def step(self):
                    pj, tt = self.pj, self.tt
                    oo, h = divmod(self.step_i, HPG)
                    if h == 0:
                        self.ps = cps.tile(
                            [128, QT], dt.float32, tag="cps",
                            name=f"cps_{pj}_{tt}_{oo}",
                        )
                        if oo == 0:
                            self.dr = cdrp.tile(
                                [128, 4, QT], dt.bfloat16, tag="cdr",
                                name=f"cdr_{pj}_{tt}",
                            )
                    nc.tensor.matmul(
                        self.ps,
                        self.o_heads[h][:, tt, :],
                        wo_sb[:, h, oo * QT:(oo + 1) * QT],
                        start=(h == 0), stop=(h == HPG - 1),
                    )
                    self.step_i += 1
                    if h == HPG - 1:
                        eng = nc.scalar if (oo % 2 == 0) else nc.vector
                        if oo % 2 == 0:
                            nc.scalar.copy(out=self.dr[:, oo, :], in_=self.ps)
                        else:
                            nc.vector.tensor_copy(out=self.dr[:, oo, :], in_=self.ps)
                        if oo == D // QT - 1:
                            nc.sync.dma_start(
                                out=out_d.ap()[
                                    pj * QT + tt * 128: pj * QT + (tt + 1) * 128, :
                                ].rearrange("p (oo f) -> p oo f", f=QT),
                                in_=self.dr,
                            )
                            return True
                    return False

            # ---------------- filler scheduler ------------------------------
            from collections import deque
            work = deque()

            def fill(n):
                for _ in range(n):
                    if not work:
                        return
                    if work[0].step():
                        work.popleft()

            def flush_A(tb):
                # emit everything still pending up to and including A(tb) units
                while any(isinstance(u, AUnit) and u.tb <= tb for u in work):
                    if work[0].step():
                        work.popleft()

            def queue_A(tb):
                for f in (0, 4, 1, 5, 2, 6, 3, 7):
                    work.append(AUnit(tb, "qk", f))
                for ts4 in range(4):
                    work.append(AUnit(tb, "v", ts4))

            # ---------------- attention head --------------------------------
            def att_head(j, h, o_heads):
                q_t = q_ts[(j, h)]
                nkc = 4 * (j + 1)

                def col0(kc):
                    m = kc - 4 * j
                    return 0 if m <= 0 else 128 * m

                def s_matmul(kc):
                    c0 = col0(kc)
                    psum_s = ps_s.tile(
                        [128, QT], dt.float32, name=f"s_{j}_{h}_{kc}", tag="psum_s",
                    )
                    nc.tensor.matmul(
                        psum_s[:, c0:],
                        k_rs[h][:, kc * 128:(kc + 1) * 128],
                        q_t[:, c0:],
                        start=True, stop=True,
                    )
                    return psum_s

                # S + exp for all chunks; pt tiles persist across the head
                pts = []
                s_next = s_matmul(0)
                for kc in range(nkc):
                    psum_s = s_next
                    if kc + 1 < nkc:
                        s_next = s_matmul(kc + 1)
                    fill(1)
                    c0 = col0(kc)
                    m = kc - 4 * j
                    pt = ptp.tile([128, QT], dt.bfloat16, tag="pt", bufs=16,
                                  name=f"pt_{j}_{h}_{kc}")
                    nc.scalar.activation(
                        out=pt[:, c0:], in_=psum_s[:, c0:], func=AF.Exp,
                        scale=SCALE,
                    )
                    if m >= 0:
                        nc.vector.tensor_mul(
                            out=pt[:, c0:c0 + 128], in0=pt[:, c0:c0 + 128],
                            in1=mask_t,
                        )
                    pts.append(pt)

                # one accumulation group per PSUM bank (A/B ping-pong): an
                # interleaved second group's start corrupts the first in-bank
                rcp = rcpp.tile([128, 4], dt.float32, tag="rcp")
                o_sb = osbp.tile([128, 4, 128], dt.bfloat16, tag="osb")
                for s in range(4):
                    pool = poaA if s % 2 == 0 else poaB
                    oa = pool.tile([128, 129], dt.float32,
                                   tag=f"oa{'AB'[s % 2]}",
                                   name=f"oa_{j}_{h}_{s}")
                    last = 4 * j + s
                    for kc in range(last + 1):
                        nc.tensor.matmul(
                            oa,
                            pts[kc][:, s * 128:(s + 1) * 128],
                            v_aug[h][:, kc, :],
                            start=(kc == 0), stop=(kc == last),
                        )
                        if kc % 2 == 0:
                            fill(1)
                    nc.vector.reciprocal_approx_fast(
                        rcp[:, s:s + 1], oa[:, 128:129]
                    )
                    nc.vector.tensor_scalar_mul(
                        o_sb[:, s, :], oa[:, 0:128], rcp[:, s:s + 1]
                    )
                work.append(TransUnit(j, h, o_sb, o_heads))

            class TransUnit:
                """Deferred [q,d] -> [d,q] flip of a head's normalized output:
                4 PE transposes + 1 ACT copy, drip-fed as filler."""

                def __init__(self, j, h, o_sb, o_heads):
                    self.j, self.h, self.o_sb, self.o_heads = j, h, o_sb, o_heads
                    self.s = 0
                    self.ptr = None

                def step(self):
                    if self.ptr is None:
                        self.ptr = ptrp.tile([128, 4, 128], dt.bfloat16,
                                             tag="ptr",
                                             name=f"ptr_{self.j}_{self.h}")
                    s = self.s
                    nc.tensor.matmul(
                        self.ptr[:, s, :], self.o_sb[:, s, :], idn_t,
                        is_transpose=True,
                    )
                    self.s += 1
                    if self.s < 4:
                        return False
                    o_h = ohp.tile([128, 4, 128], dt.bfloat16, tag=f"oh{self.h}",
                                   name=f"oh_{self.j}_{self.h}")
                    nc.scalar.copy(out=o_h, in_=self.ptr)
                    self.o_heads[self.h] = o_h
                    return True

            # ---------------- main schedule ---------------------------------
            load_x(0)
            queue_A(0)
            flush_A(0)

            prev_o = None
            for j in range(NQT):
                if j + 1 < NTB:
                    load_x(j + 1)
                    queue_A(j + 1)
                o_heads = [None] * HPG
                for h in range(HPG):
                    att_head(j, h, o_heads)
                    if prev_o is not None:
                        work.append(CUnit(j - 1, h, prev_o))
                if j + 1 < NTB:
                    flush_A(j + 1)
                prev_o = o_heads
            # flush all remaining deferred work (incl. TransUnits of j=3)
            # BEFORE the tail out-projection reads o_heads
            fill(10 ** 9)
            # tail: out-projection of the last q-tile
            for tt in range(4):
                u = CUnit(NQT - 1, tt, prev_o)
                while not u.step():
                    pass
    nc.compile()
    return nc


# ---------------------------------------------------------------------------
# Host side
# ---------------------------------------------------------------------------

_DEINT = np.concatenate([np.arange(0, HD, 2), np.arange(1, HD, 2)])  # de-interleave


def _rope_tables():
    half = HD // 2
    inv_freq = 1.0 / (ROPE_BASE ** (np.arange(half, dtype=np.float64) / half))
    t = np.arange(T, dtype=np.float64)
    fr = t[None, :] * inv_freq[:, None]          # (64, T)
    cos = np.concatenate([np.cos(fr), np.cos(fr)], axis=0).astype(np.float16)
    # halves swapped: row i<64 holds +sin(f_i) (used for rot[64+i]),
    # row 64+i holds -sin(f_i) (used for rot[i])
    sin = np.concatenate([np.sin(fr), -np.sin(fr)], axis=0).astype(np.float16)
    return cos, sin


def _mask():
    kk = np.arange(128)[:, None]
    qq = np.arange(128)[None, :]
    return (kk <= qq).astype(np.float32)


def make_in_maps(x, Wqkv, bqkv, Wo, bo):
    cos, sin = _rope_tables()
    mask = _mask()
    idn = np.eye(128, dtype=BF16)

    Wq = Wqkv[0 * D:1 * D]
    Wk = Wqkv[1 * D:2 * D]
    Wv = Wqkv[2 * D:3 * D]
    bq = bqkv[0 * D:1 * D]
    bk = bqkv[1 * D:2 * D]
    bv = bqkv[2 * D:3 * D]

    in_maps = []
    for c in range(N_CORES):
        b, g = divmod(c, GROUPS)
        hsl = slice(g * HPG * HD, (g + 1) * HPG * HD)
        # de-interleaved row order for q,k heads of this group
        rows = np.arange(g * HPG * HD, (g + 1) * HPG * HD).reshape(HPG, HD)
        rows = rows[:, _DEINT].reshape(-1)

        wq = Wq[rows]                       # (512, D)
        wk = Wk[rows]
        wv = Wv[hsl]                        # natural order
        wqkT = np.concatenate([wq, wk], axis=0).T  # (D, 1024)
        wqpack = np.ascontiguousarray(
            wqkT.reshape(NCC, 128, 8, 128)      # (cc, p, fb, f)
                .transpose(2, 1, 0, 3)           # (fb, p, cc, f)
                .reshape(8, 128, NCC * 128)
        ).astype(BF16)
        wvT = wv.T                               # (D, 512)
        wvpack = np.ascontiguousarray(
            wvT.reshape(NCC, 128, 512).transpose(1, 0, 2).reshape(128, NCC * 512)
        ).astype(BF16)
        woT = np.ascontiguousarray(Wo[:, hsl].T).astype(BF16)  # (512, D)

        bqk = np.concatenate([bq[rows], bk[rows]]).astype(np.float32)[:, None]
        bvb = np.broadcast_to(bv[hsl].astype(np.float32), (HD, 512)).copy()

        xb = np.asarray(x[b]).astype(BF16)       # (T, D) -> pack (p, cc, t)
        xpack = np.ascontiguousarray(xb.T.reshape(NCC, 128, T).transpose(1, 0, 2))

        in_maps.append({
            "xpack": xpack,
            "wqpack": wqpack,
            "wvpack": wvpack,
            "woT": woT,
            "cosT": cos,
            "sinT": sin,
            "mask": mask,
            "bqk": bqk,
            "bvb": bvb,
            "idn": idn,
        })
    return in_maps


_NC_CACHE = {}


def _get_nc(loop=1):
    if loop not in _NC_CACHE:
        _NC_CACHE[loop] = build(loop=loop)
    return _NC_CACHE[loop]


def kernel(x, Wqkv, bqkv, Wo, bo):
    x = np.asarray(x)
    Wqkv = np.asarray(Wqkv)
    bqkv = np.asarray(bqkv)
    Wo = np.asarray(Wo)
    bo = np.asarray(bo)

    nc = _get_nc()
    in_maps = make_in_maps(x, Wqkv, bqkv, Wo, bo)
    res = run_bass_kernel_spmd(nc, in_maps, core_ids=list(range(N_CORES)))

    out = np.zeros((B, T, D), dtype=np.float32)
    for c in range(N_CORES):
        b = c // GROUPS
        out[b] += res.results[c]["outp"].astype(np.float32)
    out += bo.astype(np.float32)[None, None, :]
    return out


# revision 15
# speedup vs baseline: 1.3678x; 1.1206x over previous
"""Multi-head causal attention with RoPE for TRN2, 8 NeuronCores.

Problem: B=2, T=2048, D=2048, 16 heads x head_dim 128, fp32 in/out.
  qkv = x @ Wqkv.T + bqkv ; RoPE(q,k) interleaved-pairs; causal softmax attention;
  out = attn_out @ Wo.T + bo.

Sharding: core c in 0..7 -> (batch b = c//4, head-group g = c%4 of 4 heads).
Each core computes its batch's partial output (its 4 heads' contribution through
the out-projection); host sums the 4 group partials per batch and adds bo.

v2 design (vs the fp32r DRAM-roundtrip baseline):
  - bf16 datapath everywhere on PE (weights, x, q, k, v, pt, O, Wo); fp32 PSUM.
  - No DRAM roundtrips: q/k/v drain from PSUM directly into SBUF in the exact
    layout attention needs (q tiles ARE [head, 512-q-tile] blocks; v tiles ARE
    [128 kpos, kc, head*128..] slices of the projection drain).
  - Phase A (projection) and attention pipelined per 512-token t-block:
    causal q-tile j only needs k,v up to (j+1)*512, i.e. t-blocks <= j.
    A-unit matmuls are drip-fed into attention's PE idle slots (attention alone
    is ACT(exp)-bound; PE has ~200ns/chunk spare).
  - RoPE rotate-half via partition-offset DVE muls (no PE perm matmul).
  - Softmax denominator for free: v is stored with a ones-column appended
    ([128, kc, 129]); attention output is computed TRANSPOSED per 128-wide
    q-sub-block (stationary = pt slice, moving = v_aug) so each PV matmul
    emits [q, 128 d + denom] in one pass. Normalization is then a per-partition
    tensor_scalar_mul with reciprocal_approx_fast on [128,4] (cheap), and the
    [q,d] -> [d,q] flip for the out-projection is 4 tiny PE transposes.
"""
import os
import sys

for _p in ("/opt/trn_rl_repo", "/root/.axon_site/_ro/trn_rl_repo"):
    if os.path.isdir(_p) and _p not in sys.path:
        sys.path.insert(0, _p)

import numpy as np
import ml_dtypes

import concourse.bacc as bacc
import concourse.mybir as mybir
import concourse.tile as tile
from concourse.bass_utils import run_bass_kernel_spmd

dt = mybir.dt
AF = mybir.ActivationFunctionType
BF16 = ml_dtypes.bfloat16

B = 2
T = 2048
D = 2048
NH = 16
HD = 128
ROPE_BASE = 10000.0
N_CORES = 8
GROUPS = 4          # head-groups (tensor-parallel axis)
HPG = NH // GROUPS  # heads per group = 4
QT = 512            # q-tile width in attention
NQT = T // QT       # 4
NCC = D // 128      # 16 contraction chunks
TB = 512            # phase-A t-block == QT
NTB = T // TB       # 4
SCALE = 1.0 / float(np.sqrt(HD))


def build(loop=1):
    """Emit the per-core BIR program (identical for all 8 cores)."""
    import contextlib

    nc = bacc.Bacc("TRN2", target_bir_lowering=False, debug=False)

    xp_d = nc.dram_tensor("xpack", [128, NCC, T], dt.bfloat16, kind="ExternalInput")
    wqp_d = nc.dram_tensor("wqpack", [8, 128, NCC * 128], dt.bfloat16,
                           kind="ExternalInput")
    wvp_d = nc.dram_tensor("wvpack", [128, NCC * 512], dt.bfloat16,
                           kind="ExternalInput")
    woT_d = nc.dram_tensor("woT", [HPG * HD, D], dt.bfloat16, kind="ExternalInput")
    cos_d = nc.dram_tensor("cosT", [HD, T], dt.float16, kind="ExternalInput")
    sin_d = nc.dram_tensor("sinT", [HD, T], dt.float16, kind="ExternalInput")
    mask_d = nc.dram_tensor("mask", [128, 128], dt.float32, kind="ExternalInput")
    bqk_d = nc.dram_tensor("bqk", [8 * 128, 1], dt.float32, kind="ExternalInput")
    bv_d = nc.dram_tensor("bvb", [HD, 512], dt.float32, kind="ExternalInput")
    idn_d = nc.dram_tensor("idn", [128, 128], dt.bfloat16, kind="ExternalInput")
    out_d = nc.dram_tensor("outp", [T, D], dt.bfloat16, kind="ExternalOutput")

    with tile.TileContext(nc, pool_alloc_mode="queue") as tc:
        with contextlib.ExitStack() as ctx:
            if loop > 1:
                ctx.enter_context(tc.For_i(0, loop, 1))
            P = lambda *a, **kw: ctx.enter_context(tc.tile_pool(*a, **kw))
            kres = P(name="kres", bufs=1)
            wpool = P(name="wq", bufs=1)
            xpool = P(name="xb", bufs=1)
            s1p = P(name="s1p", bufs=2)
            rotp = P(name="rotp", bufs=2)
            csts = P(name="csts", bufs=1)
            qres = P(name="qres", bufs=1)
            vres = P(name="vres", bufs=1)
            ptp = P(name="ptp", bufs=4)
            osbp = P(name="osb", bufs=4)
            ohp = P(name="ohp", bufs=2)
            rcpp = P(name="rcpp", bufs=2)
            cdrp = P(name="cdr", bufs=2)
            aps = P(name="aps", bufs=2, space="PSUM")
            ps_s = P(name="ps_s", bufs=2, space="PSUM")
            poaA = P(name="poaA", bufs=1, space="PSUM")
            poaB = P(name="poaB", bufs=1, space="PSUM")
            ptrp = P(name="ptr", bufs=1, space="PSUM")
            cps = P(name="cps", bufs=1, space="PSUM")
            # ---------------- constant / weight loads (scalar HWDGE) --------
            wq_blocks = [None] * 8
            def load_wq(f):
                wq_b = wpool.tile([128, NCC, 128], dt.bfloat16, tag=f"wq{f}",
                                  name=f"wq_{f}")
                nc.gpsimd.dma_start(
                    out=wq_b,
                    in_=wqp_d.ap()[f].rearrange("p (cc f) -> p cc f", f=128),
                )
                wq_blocks[f] = wq_b

            load_wq(0)
            load_wq(4)
            bqk_sb = csts.tile([128, 8, 1], dt.float32)
            nc.gpsimd.dma_start(
                out=bqk_sb, in_=bqk_d.ap().rearrange("(f p) o -> p f o", p=128)
            )
            cos_t = csts.tile([HD, T], dt.float16)
            sin_t = csts.tile([HD, T], dt.float16)
            nc.gpsimd.dma_start(out=cos_t, in_=cos_d.ap())
            nc.gpsimd.dma_start(out=sin_t, in_=sin_d.ap())
            load_wq(1)
            load_wq(5)
            bv_sb = csts.tile([HD, 512], dt.float32)
            nc.gpsimd.dma_start(out=bv_sb, in_=bv_d.ap())
            mask_t = csts.tile([128, 128], dt.float32)
            nc.gpsimd.dma_start(out=mask_t, in_=mask_d.ap())
            idn_t = csts.tile([128, 128], dt.bfloat16)
            nc.gpsimd.dma_start(out=idn_t, in_=idn_d.ap())
            load_wq(2)
            load_wq(6)
            load_wq(3)
            load_wq(7)
            wv_b = wpool.tile([128, NCC, 512], dt.bfloat16)
            nc.gpsimd.dma_start(
                out=wv_b, in_=wvp_d.ap().rearrange("p (cc f) -> p cc f", f=512),
            )
            wo_sb = wpool.tile([128, HPG, D], dt.bfloat16)
            nc.gpsimd.dma_start(
                out=wo_sb, in_=woT_d.ap().rearrange("(hh p) o -> p hh o", p=128)
            )

            # ---------------- persistent q/k/v ------------------------------
            k_rs = [kres.tile([HD, T], dt.bfloat16, tag=f"kr{h}", name=f"kr_{h}")
                    for h in range(HPG)]
            q_ts = {}
            for tb in range(NTB):
                for h in range(HPG):
                    q_ts[(tb, h)] = qres.tile([HD, QT], dt.bfloat16,
                                              tag=f"q{tb}_{h}", name=f"q_{tb}_{h}")
            v_aug = [vres.tile([128, 4 * NTB, 129], dt.bfloat16, tag=f"v{h}",
                               name=f"v_{h}") for h in range(HPG)]
            for h in range(HPG):
                nc.gpsimd.memset(v_aug[h][:, :, 128], 1.0)

            # ---------------- x slab loads (sync HWDGE) ---------------------
            x_slabs = {}

            def load_x(tb):
                tsl = slice(tb * TB, (tb + 1) * TB)
                if tb == 0:
                    parts = []
                    for qr in range(4):
                        xq = xpool.tile([128, 4, TB], dt.bfloat16, tag=f"x0q{qr}",
                                        name=f"x0_{qr}", bufs=1)
                        nc.sync.dma_start(
                            out=xq, in_=xp_d.ap()[:, 4 * qr:4 * (qr + 1), tsl]
                        )
                        parts.append(xq)
                    x_slabs[0] = ("quads", parts)
                else:
                    xs = xpool.tile([128, NCC, TB], dt.bfloat16, tag="xslab",
                                    name=f"x_{tb}")
                    nc.sync.dma_start(out=xs, in_=xp_d.ap()[:, :, tsl])
                    x_slabs[tb] = ("slab", xs)

            def x_chunk(tb, cc):
                kind, v = x_slabs[tb]
                if kind == "quads":
                    return v[cc // 4][:, cc % 4, :]
                return v[:, cc, :]

            # ---------------- phase-A unit (drip-fed into PE stream) --------
            class AUnit:
                """One projection unit: 16 accumulating matmuls + drain.

                kind 'qk': f-block f (0-3 = q heads, 4-7 = k heads), out RoPE'd.
                kind 'v': ts4 sub-block, out v_aug slices (+bias).
                """

                def __init__(self, tb, kind, idx):
                    self.tb, self.kind, self.idx = tb, kind, idx
                    self.cc = 0
                    self.ps = None

                def step(self):
                    """Emit one PE matmul; returns True when unit is done."""
                    tb, kind, idx = self.tb, self.kind, self.idx
                    if self.ps is None:
                        self.ps = aps.tile(
                            [128, TB], dt.float32, tag="aps",
                            name=f"aps_{tb}_{kind}{idx}",
                        )
                    cc = self.cc
                    if kind == "qk":
                        nc.tensor.matmul(
                            self.ps, wq_blocks[idx][:, cc, :], x_chunk(tb, cc),
                            start=(cc == 0), stop=(cc == NCC - 1),
                        )
                    else:
                        nc.tensor.matmul(
                            self.ps,
                            x_chunk(tb, cc)[:, idx * 128:(idx + 1) * 128],
                            wv_b[:, cc, :],
                            start=(cc == 0), stop=(cc == NCC - 1),
                        )
                    self.cc += 1
                    if self.cc < NCC:
                        return False
                    self._drain()
                    return True

                def _drain(self):
                    tb, kind, idx = self.tb, self.kind, self.idx
                    tsl = slice(tb * TB, (tb + 1) * TB)
                    if kind == "qk":
                        f = idx
                        s1 = s1p.tile([128, TB], dt.bfloat16, tag="s1")
                        nc.vector.tensor_scalar_add(s1, self.ps, bqk_sb[:, f, :])
                        rot = rotp.tile([128, TB], dt.bfloat16, tag="rot")
                        # sin table halves are pre-swapped host-side so both
                        # SBUF inputs share a base partition (HW constraint)
                        nc.vector.tensor_mul(
                            out=rot[0:64, :], in0=s1[64:128, :],
                            in1=sin_t[64:128, tsl],
                        )
                        nc.vector.tensor_mul(
                            out=rot[64:128, :], in0=s1[0:64, :],
                            in1=sin_t[0:64, tsl],
                        )
                        nc.vector.tensor_mul(out=s1, in0=s1, in1=cos_t[:, tsl])
                        if f < 4:
                            nc.vector.tensor_add(out=q_ts[(tb, f)], in0=s1, in1=rot)
                        else:
                            nc.vector.tensor_add(
                                out=k_rs[f - 4][:, tsl], in0=s1, in1=rot
                            )
                    else:
                        kc = 4 * tb + idx
                        for h in range(HPG):
                            nc.vector.tensor_add(
                                out=v_aug[h][:, kc, 0:128],
                                in0=self.ps[:, h * 128:(h + 1) * 128],
                                in1=bv_sb[:, h * 128:(h + 1) * 128],
                            )

            # ---------------- cproj unit (out-projection, drip-fed) ---------
            class CUnit:
                """Out-projection for q-tile pj, row-block tt: 4 oo groups of
                4 accumulating matmuls, drained to one [128, 4*512] bf16 tile,
                then one DMA."""

                def __init__(self, pj, tt, o_heads):
                    self.pj, self.tt, self.o_heads = pj, tt, o_heads
                    self.step_i = 0
                    self.ps = None
                    self.dr = None

                def step(self):
                    pj, tt = self.pj, self.tt
                    oo, h = divmod(self.step_i, HPG)
                    if h == 0:
                        self.ps = cps.tile(
                            [128, QT], dt.float32, tag="cps",
                            name=f"cps_{pj}_{tt}_{oo}",
                        )
                        if oo == 0:
                            self.dr = cdrp.tile(
                                [128, 4, QT], dt.bfloat16, tag="cdr",
                                name=f"cdr_{pj}_{tt}",
                            )
                    nc.tensor.matmul(
                        self.ps,
                        self.o_heads[h][:, tt, :],
                        wo_sb[:, h, oo * QT:(oo + 1) * QT],
                        start=(h == 0), stop=(h == HPG - 1),
                    )
                    self.step_i += 1
                    if h == HPG - 1:
                        eng = nc.scalar if (oo % 2 == 0) else nc.vector
                        if oo % 2 == 0:
                            nc.scalar.copy(out=self.dr[:, oo, :], in_=self.ps)
                        else:
                            nc.vector.tensor_copy(out=self.dr[:, oo, :], in_=self.ps)
                        if oo == D // QT - 1:
                            nc.gpsimd.dma_start(
                                out=out_d.ap()[
                                    pj * QT + tt * 128: pj * QT + (tt + 1) * 128, :
                                ].rearrange("p (oo f) -> p oo f", f=QT),
                                in_=self.dr,
                            )
                            return True
                    return False

            # ---------------- filler scheduler ------------------------------
            from collections import deque
            work = deque()

            def fill(n):
                for _ in range(n):
                    if not work:
                        return
                    if work[0].step():
                        work.popleft()

            def flush_A(tb):
                # emit everything still pending up to and including A(tb) units
                while any(isinstance(u, AUnit) and u.tb <= tb for u in work):
                    if work[0].step():
                        work.popleft()

            def queue_A(tb):
                for f in (0, 4, 1, 5, 2, 6, 3, 7):
                    work.append(AUnit(tb, "qk", f))
                for ts4 in range(4):
                    work.append(AUnit(tb, "v", ts4))

            # ---------------- attention head --------------------------------
            def att_head(j, h, o_heads):
                q_t = q_ts[(j, h)]
                nkc = 4 * (j + 1)

                def col0(kc):
                    m = kc - 4 * j
                    return 0 if m <= 0 else 128 * m

                def s_matmul(kc):
                    c0 = col0(kc)
                    psum_s = ps_s.tile(
                        [128, QT], dt.float32, name=f"s_{j}_{h}_{kc}", tag="psum_s",
                    )
                    nc.tensor.matmul(
                        psum_s[:, c0:],
                        k_rs[h][:, kc * 128:(kc + 1) * 128],
                        q_t[:, c0:],
                        start=True, stop=True,
                    )
                    return psum_s

                # S + exp for all chunks; pt tiles persist across the head
                pts = []
                s_next = s_matmul(0)
                for kc in range(nkc):
                    psum_s = s_next
                    if kc + 1 < nkc:
                        s_next = s_matmul(kc + 1)
                    fill(1)
                    c0 = col0(kc)
                    m = kc - 4 * j
                    pt = ptp.tile([128, QT], dt.bfloat16, tag="pt", bufs=16,
                                  name=f"pt_{j}_{h}_{kc}")
                    nc.scalar.activation(
                        out=pt[:, c0:], in_=psum_s[:, c0:], func=AF.Exp,
                        scale=SCALE,
                    )
                    if m >= 0:
                        nc.vector.tensor_mul(
                            out=pt[:, c0:c0 + 128], in0=pt[:, c0:c0 + 128],
                            in1=mask_t,
                        )
                    pts.append(pt)

                # one accumulation group per PSUM bank (A/B ping-pong): an
                # interleaved second group's start corrupts the first in-bank
                rcp = rcpp.tile([128, 4], dt.float32, tag="rcp")
                o_sb = osbp.tile([128, 4, 128], dt.bfloat16, tag="osb")
                for s in range(4):
                    pool = poaA if s % 2 == 0 else poaB
                    oa = pool.tile([128, 129], dt.float32,
                                   tag=f"oa{'AB'[s % 2]}",
                                   name=f"oa_{j}_{h}_{s}")
                    last = 4 * j + s
                    for kc in range(last + 1):
                        nc.tensor.matmul(
                            oa,
                            pts[kc][:, s * 128:(s + 1) * 128],
                            v_aug[h][:, kc, :],
                            start=(kc == 0), stop=(kc == last),
                        )
                        if kc % 2 == 0:
                            fill(1)
                    nc.vector.reciprocal_approx_fast(
                        rcp[:, s:s + 1], oa[:, 128:129]
                    )
                    nc.vector.tensor_scalar_mul(
                        o_sb[:, s, :], oa[:, 0:128], rcp[:, s:s + 1]
                    )
                work.append(TransUnit(j, h, o_sb, o_heads))

            class TransUnit:
                """Deferred [q,d] -> [d,q] flip of a head's normalized output:
                4 PE transposes + 1 ACT copy, drip-fed as filler."""

                def __init__(self, j, h, o_sb, o_heads):
                    self.j, self.h, self.o_sb, self.o_heads = j, h, o_sb, o_heads
                    self.s = 0
                    self.ptr = None

                def step(self):
                    if self.ptr is None:
                        self.ptr = ptrp.tile([128, 4, 128], dt.bfloat16,
                                             tag="ptr",
                                             name=f"ptr_{self.j}_{self.h}")
                    s = self.s
                    nc.tensor.matmul(
                        self.ptr[:, s, :], self.o_sb[:, s, :], idn_t,
                        is_transpose=True,
                    )
                    self.s += 1
                    if self.s < 4:
                        return False
                    o_h = ohp.tile([128, 4, 128], dt.bfloat16, tag=f"oh{self.h}",
                                   name=f"oh_{self.j}_{self.h}")
                    nc.scalar.copy(out=o_h, in_=self.ptr)
                    self.o_heads[self.h] = o_h
                    return True

            # ---------------- main schedule ---------------------------------
            load_x(0)
            queue_A(0)
            flush_A(0)

            prev_o = None
            for j in range(NQT):
                if j + 1 < NTB:
                    load_x(j + 1)
                    queue_A(j + 1)
                o_heads = [None] * HPG
                for h in range(HPG):
                    att_head(j, h, o_heads)
                    if prev_o is not None:
                        work.append(CUnit(j - 1, h, prev_o))
                if j + 1 < NTB:
                    flush_A(j + 1)
                prev_o = o_heads
            # flush all remaining deferred work (incl. TransUnits of j=3)
            # BEFORE the tail out-projection reads o_heads
            fill(10 ** 9)
            # tail: out-projection of the last q-tile
            for tt in range(4):
                u = CUnit(NQT - 1, tt, prev_o)
                while not u.step():
                    pass
    nc.compile()
    return nc


# ---------------------------------------------------------------------------
# Host side
# ---------------------------------------------------------------------------

_DEINT = np.concatenate([np.arange(0, HD, 2), np.arange(1, HD, 2)])  # de-interleave


def _rope_tables():
    half = HD // 2
    inv_freq = 1.0 / (ROPE_BASE ** (np.arange(half, dtype=np.float64) / half))
    t = np.arange(T, dtype=np.float64)
    fr = t[None, :] * inv_freq[:, None]          # (64, T)
    cos = np.concatenate([np.cos(fr), np.cos(fr)], axis=0).astype(np.float16)
    # halves swapped: row i<64 holds +sin(f_i) (used for rot[64+i]),
    # row 64+i holds -sin(f_i) (used for rot[i])
    sin = np.concatenate([np.sin(fr), -np.sin(fr)], axis=0).astype(np.float16)
    return cos, sin


def _mask():
    kk = np.arange(128)[:, None]
    qq = np.arange(128)[None, :]
    return (kk <= qq).astype(np.float32)


def make_in_maps(x, Wqkv, bqkv, Wo, bo):
    cos, sin = _rope_tables()
    mask = _mask()
    idn = np.eye(128, dtype=BF16)

    Wq = Wqkv[0 * D:1 * D]
    Wk = Wqkv[1 * D:2 * D]
    Wv = Wqkv[2 * D:3 * D]
    bq = bqkv[0 * D:1 * D]
    bk = bqkv[1 * D:2 * D]
    bv = bqkv[2 * D:3 * D]

    in_maps = []
    for c in range(N_CORES):
        b, g = divmod(c, GROUPS)
        hsl = slice(g * HPG * HD, (g + 1) * HPG * HD)
        # de-interleaved row order for q,k heads of this group
        rows = np.arange(g * HPG * HD, (g + 1) * HPG * HD).reshape(HPG, HD)
        rows = rows[:, _DEINT].reshape(-1)

        wq = Wq[rows]                       # (512, D)
        wk = Wk[rows]
        wv = Wv[hsl]                        # natural order
        wqkT = np.concatenate([wq, wk], axis=0).T  # (D, 1024)
        wqpack = np.ascontiguousarray(
            wqkT.reshape(NCC, 128, 8, 128)      # (cc, p, fb, f)
                .transpose(2, 1, 0, 3)           # (fb, p, cc, f)
                .reshape(8, 128, NCC * 128)
        ).astype(BF16)
        wvT = wv.T                               # (D, 512)
        wvpack = np.ascontiguousarray(
            wvT.reshape(NCC, 128, 512).transpose(1, 0, 2).reshape(128, NCC * 512)
        ).astype(BF16)
        woT = np.ascontiguousarray(Wo[:, hsl].T).astype(BF16)  # (512, D)

        bqk = np.concatenate([bq[rows], bk[rows]]).astype(np.float32)[:, None]
        bvb = np.broadcast_to(bv[hsl].astype(np.float32), (HD, 512)).copy()

        xb = np.asarray(x[b]).astype(BF16)       # (T, D) -> pack (p, cc, t)
        xpack = np.ascontiguousarray(xb.T.reshape(NCC, 128, T).transpose(1, 0, 2))

        in_maps.append({
            "xpack": xpack,
            "wqpack": wqpack,
            "wvpack": wvpack,
            "woT": woT,
            "cosT": cos,
            "sinT": sin,
            "mask": mask,
            "bqk": bqk,
            "bvb": bvb,
            "idn": idn,
        })
    return in_maps


_NC_CACHE = {}


def _get_nc(loop=1):
    if loop not in _NC_CACHE:
        _NC_CACHE[loop] = build(loop=loop)
    return _NC_CACHE[loop]


def kernel(x, Wqkv, bqkv, Wo, bo):
    x = np.asarray(x)
    Wqkv = np.asarray(Wqkv)
    bqkv = np.asarray(bqkv)
    Wo = np.asarray(Wo)
    bo = np.asarray(bo)

    nc = _get_nc()
    in_maps = make_in_maps(x, Wqkv, bqkv, Wo, bo)
    res = run_bass_kernel_spmd(nc, in_maps, core_ids=list(range(N_CORES)))

    out = np.zeros((B, T, D), dtype=np.float32)
    for c in range(N_CORES):
        b = c // GROUPS
        out[b] += res.results[c]["outp"].astype(np.float32)
    out += bo.astype(np.float32)[None, None, :]
    return out
